# revision 10
# baseline (speedup 1.0000x reference)
"""Trainium2 Bass kernel for FISTA sparse coding (nn_FISTA_7550552506950).

Strategy (data-parallel over batch, 8 cores x 128 rows):
- State z kept TRANSPOSED [F=4096, B=128] on-chip as a single fp32 tensor,
  split into 32 f-chunks of [128, 256] (real|imag column halves). Everything
  stays SBUF/PSUM resident across all 25 FISTA iterations; HBM traffic is
  only the initial weight/x load and the final magnitude store.
- Precision plan (validated by host-side quantization ablation):
  * z state and the soft-threshold scale factor stay fp32 (their
    quantization compounds O(k^2) through the momentum recursion);
  * the residual R4 and the gradient weights W2a/W2b are fp16, scaled by
    1/64 so values sit mid-range (fp16 matmuls stream ~1.0 cyc/col vs
    fp32r's ~1.1 + 111ns instruction floor);
  * the threshold pipeline t12/m2 is fp16 SCALED by K=1/(2*thr): unscaled
    |u|^2 ~ 1e-8 sits in fp16's subnormal range and destroys convergence.
    q = Rsqrt(4*m2') = thr/|u| exactly, computed via the raw-Rsqrt escape.
- Per iteration the tensor engine does 40960 column-cycles: 16384 momentum
  (fp32r scaled-identity matmuls folding w = a*z + b*z_old into the PSUM
  accumulation), 16384 gradient (fp16), 8192 A-chain (fp32r).
- Soft-threshold per group: Square on ACT (scale K, fp16 out), m2 pair-add
  on DVE fp16 tensor_tensor (~0.33us vs 1.5us on GPSIMD -- this removes
  GPSIMD from the iteration-boundary critical chain entirely), Rsqrt on
  ACT (scale 4, fp32 out), ns = min(q,1)-1 on DVE, z = (u * -1) * ns as
  DVE scalar_tensor_tensor per half.
- The PE clock (HAM gate) halves after a ~3.4us idle window, so the
  schedule keeps every PE gap short: tapered groups [2,4,...,4,2],
  just-in-time momentum emission, deferred A-chain interleaving, and the
  next iteration's first momentum groups filling the end-of-iteration
  drain. The b-part of the residual combo is precomputed at iteration
  start; R4ns is built by DVE ts ops (fp16) right after the R4 combo.
- P1 products live in a persistent 2-slot (1 PSUM bank) tile; readers of
  the previous slot are ordered before the bank clear via DVE program
  order (see emit order comments).
- Iteration 0 (w = 0) skips all momentum matmuls; iteration 1 (gamma = 0)
  skips the b-part. Global max normalization happens on host.
"""

import numpy as np
from contextlib import ExitStack

import concourse.bass as bass
import concourse.mybir as mybir
import concourse.tile as tile
from concourse import bacc
from concourse.bass_utils import run_bass_kernel_spmd

F32 = mybir.dt.float32
F32R = mybir.dt.float32r
F16 = mybir.dt.float16
ALU = mybir.AluOpType
ACTF = mybir.ActivationFunctionType

P = 128          # partitions / f-chunk size
F = 4096         # dictionary size
T = 64           # signal dim
NCH = F // P     # 32 chunks
B = 128          # batch rows per core
NCORES = 8
MAX_ITER = 25
STEP = np.float32(1.0 / F)
THR = np.float32(0.5) * STEP
SC = np.float32(1.0 / 64)          # residual scaling (keeps fp16 mid-range)
KSQ = float(1.0 / (2.0 * float(THR)))   # Square scale: t12 = (u*K)^2
MAGS = float(4.0 * float(THR))          # |u| = 4*thr * m2' * q
DEFER_CHUNKS = 12   # A-chain chunks deferred behind the threshold pipeline
NS_ON_ACT = set()   # groups whose ns runs on ACT (engine balance)

# group taper: (c0, n) — 2 chunks first (early PSUM release), 2 last (short
# drain chain); 4-chunk groups in between
GROUPS = [(0, 2)] + [(2 + 4 * i, 4) for i in range(7)] + [(30, 2)]


def _activation_raw(nc, out, in_, func, bias, scale=1.0):
    """nc.scalar.activation minus the Rsqrt accuracy guard.

    Safe here: rsqrt feeds the soft-threshold scale factor (error attenuated
    by thr/mag) and the final magnitude (relative error ~1e-3, far inside
    the 2e-2 gate).
    """
    inputs = [nc.scalar.lower_ap(in_)]
    for arg in (bias, scale, 0.0):
        if isinstance(arg, float):
            inputs.append(mybir.ImmediateValue(dtype=F32, value=arg))
        else:
            inputs.append(nc.scalar.lower_ap(arg))
    return nc.scalar.add_instruction(
        mybir.InstActivation(
            name=nc.get_next_instruction_name(),
            func=func,
            ins=inputs,
            outs=[nc.scalar.lower_ap(out)],
        )
    )


def _momentum_scalars():
    ts_ = [1.0]
    for _ in range(MAX_ITER + 1):
        ts_.append((1.0 + np.sqrt(1.0 + 4.0 * ts_[-1] ** 2)) / 2.0)
    alphas, betas = [], []
    for j in range(1, MAX_ITER + 1):
        gam = 0.0 if j <= 2 else (ts_[j - 2] - 1.0) / ts_[j - 1]
        alphas.append(float(np.float32(1.0 + gam)))
        betas.append(float(np.float32(-gam)))
    return alphas, betas


def build_nc():
    nc = bacc.Bacc(None)
    W1_d = nc.declare_dram_parameter("W1", [P, NCH, P], F32R, isOutput=False)
    W2a_d = nc.declare_dram_parameter("W2a", [P, NCH, P], F16, isOutput=False)
    W2b_d = nc.declare_dram_parameter("W2b", [P, NCH, P], F16, isOutput=False)
    XcN_d = nc.declare_dram_parameter("XcN", [P, 2 * B], F16, isOutput=False)
    Rns0_d = nc.declare_dram_parameter("Rns0", [P, 2 * B], F16, isOutput=False)
    idn_d = nc.declare_dram_parameter("idn", [P, P], F32, isOutput=False)
    mag_d = nc.declare_dram_parameter("magT", [P, NCH, B], F16, isOutput=True)

    alphas, betas = _momentum_scalars()

    with tile.TileContext(nc) as tc, ExitStack() as ctx:
        state = ctx.enter_context(tc.tile_pool(name="state", bufs=1))
        temps = ctx.enter_context(tc.tile_pool(name="temps", bufs=3))
        small = ctx.enter_context(tc.tile_pool(name="small", bufs=2))
        psum_u4 = ctx.enter_context(tc.tile_pool(name="psum_u4", bufs=3, space="PSUM"))
        psum_u2 = ctx.enter_context(tc.tile_pool(name="psum_u2", bufs=1, space="PSUM"))
        psum_s = ctx.enter_context(tc.tile_pool(name="psum_s", bufs=1, space="PSUM"))

        # ---- persistent SBUF tensors
        W1 = state.tile([P, NCH, P], F32R, tag="W1")
        W2a = state.tile([P, NCH, P], F16, tag="W2a")
        W2b = state.tile([P, NCH, P], F16, tag="W2b")
        XcN = state.tile([P, 2 * B], F16, tag="XcN")
        Rns0 = state.tile([P, 2 * B], F16, tag="Rns0")
        idn = state.tile([P, P], F32, tag="idn")
        zA = state.tile([P, NCH, 2 * B], F32R, tag="zA")
        zB = state.tile([P, NCH, 2 * B], F32R, tag="zB")
        magT = state.tile([P, NCH, B], F16, tag="magT")
        zero_col = state.tile([P, 1], F32, tag="zc")
        one_col = state.tile([P, 1], F32, tag="oc")
        eps_col = state.tile([P, 1], F32, tag="ec")

        # persistent 2-slot P1 accumulator (1 PSUM bank, slot j%2)
        P1all = psum_s.tile([P, 2, 2 * B], F32, tag="P1")

        nc.sync.dma_start(XcN[:], XcN_d[:])
        nc.sync.dma_start(Rns0[:], Rns0_d[:])
        nc.sync.dma_start(idn[:], idn_d[:])
        for k in range(8):
            cs = slice(4 * k, 4 * k + 4)
            nc.sync.dma_start(W2a[:, cs, :], W2a_d[:, cs, :])
            nc.sync.dma_start(W2b[:, cs, :], W2b_d[:, cs, :])
        nc.sync.dma_start(W1[:], W1_d[:])

        nc.vector.memset(zero_col[:], 0.0)
        nc.vector.memset(one_col[:], 1.0)
        nc.vector.memset(eps_col[:], 1e-30)

        zbuf = [zA, zB]
        pending = []     # deferred A-chain entries: (z_tile, slot, c0, n)
        u_tiles = {}     # (iteration, group-idx) -> u PSUM tile
        r4 = {0: (XcN, Rns0)}   # iteration -> (R4, R4ns), prepared one ahead
        ab_tiles = {}    # iteration -> (aI, bI)
        rb_tiles = {}    # iteration -> Rb = b*P1(z_{j-2}) + XcN, precomputed

        def emit_mom(j, gi):
            """Momentum identity matmuls for iteration j, group gi (allocates
            the group's u PSUM tile: the last group uses the 1-bank pool)."""
            c0, n = GROUPS[gi]
            if gi == len(GROUPS) - 1:
                u_ps = psum_u2.tile([P, 2, 2 * B], F32, tag="u2")
            else:
                u_ps = psum_u4.tile([P, 4, 2 * B], F32, tag="u4")
            u_tiles[(j, gi)] = u_ps
            if j == 0:
                return u_ps
            aI, bI = ab_tiles[j]
            z_prev = zbuf[(j + 1) % 2]
            z_prev2 = zbuf[j % 2]
            for pi in range(n // 2):
                c2 = c0 + 2 * pi
                out_sl = u_ps[:, 2 * pi:2 * pi + 2, :].rearrange("p c n -> p (c n)")
                nc.tensor.matmul(
                    out_sl, aI[:],
                    z_prev[:, c2:c2 + 2, :].rearrange("p c n -> p (c n)"),
                    start=True, stop=False, skip_group_check=True,
                )
                if j >= 2:
                    nc.tensor.matmul(
                        out_sl, bI[:],
                        z_prev2[:, c2:c2 + 2, :].rearrange("p c n -> p (c n)"),
                        start=False, stop=False, skip_group_check=True,
                    )
            return u_ps

        def emit_grad(j, u_ps, c0, n):
            """Gradient matmuls for chunks [c0, c0+n): all W2a first (they
            need only R4, so the next iteration's gradient starts as soon as
            the R4 combo lands; R4ns arrives while the W2a matmuls stream)."""
            R4, R4ns = r4[j]
            mom_on = j >= 1
            for i in range(n):
                nc.tensor.matmul(
                    u_ps[:, i, :], W2a[:, c0 + i, :], R4[:],
                    start=(not mom_on and i % 2 == 0),
                    stop=False, skip_group_check=True,
                )
            for i in range(n):
                nc.tensor.matmul(
                    u_ps[:, i, :], W2b[:, c0 + i, :], R4ns[:],
                    start=False, stop=(i == n - 1), skip_group_check=True,
                )

        def emit_A(entry):
            z_t, slot, c0, n = entry
            for i in range(n):
                c = c0 + i
                nc.tensor.matmul(
                    P1all[:, slot, :], W1[:, c, :], z_t[:, c, :],
                    start=(c == 0), stop=(c == NCH - 1), skip_group_check=True,
                )

        def pending_chunks():
            return sum(e[3] for e in pending)

        def emit_stage1(j, u_ps, gi, c0, n):
            """Threshold stage 1: t12 = (u*K)^2 on ACT (fp16, scaled to
            dodge fp16 subnormals) + m2 pair-add on DVE fp16."""
            u_sl = u_ps[:, 0:n, :]
            t12 = temps.tile([P, n, 2 * B], F16, tag=f"t12_{n}")
            nc.scalar.activation(t12[:], u_sl, ACTF.Square, bias=zero_col[:],
                                 scale=KSQ)
            m2 = temps.tile([P, n, B], F16, tag=f"m2_{n}")
            nc.vector.tensor_tensor(m2[:], t12[:, :, 0:B], t12[:, :, B:2 * B],
                                    ALU.add)
            return (u_ps, gi, c0, n, m2)

        def emit_stage2(j, st, z_new, last):
            """Threshold stage 2 (one group SKEWED behind stage 1 so each
            engine's queue order matches input readiness — no head-of-line
            blocking): q = thr/|u| via Rsqrt(4*m2'), ns = min(q,1)-1 = -s,
            z = (u * -1) * ns."""
            u_ps, gi, c0, n, m2 = st
            u_sl = u_ps[:, 0:n, :]
            q = temps.tile([P, n, B], F32, tag=f"q_{n}")
            _activation_raw(nc, q[:], m2[:], ACTF.Rsqrt, bias=eps_col[:],
                            scale=4.0)
            ns = temps.tile([P, n, B], F32, tag=f"ns_{n}")
            nc.vector.tensor_scalar(ns[:], q[:], 1.0, 1.0,
                                    ALU.min, ALU.subtract)

            if not last:
                z_sl = z_new[:, c0:c0 + n, :]
                nc.vector.scalar_tensor_tensor(
                    z_sl[:, :, 0:B], u_sl[:, :, 0:B], -1.0, ns[:],
                    ALU.mult, ALU.mult,
                )
                nc.vector.scalar_tensor_tensor(
                    z_sl[:, :, B:2 * B], u_sl[:, :, B:2 * B], -1.0, ns[:],
                    ALU.mult, ALU.mult,
                )
                pending.append((z_new, j % 2, c0, n))
            else:
                # |z| = |u|*s = (4*thr * m2' * q) * s  — no Sqrt needed
                tm = temps.tile([P, n, B], F32, tag=f"tm_{n}")
                nc.vector.tensor_tensor(tm[:], m2[:], q[:], ALU.mult)
                nc.vector.scalar_tensor_tensor(
                    magT[:, c0:c0 + n, :], tm[:], -MAGS,
                    ns[:], ALU.mult, ALU.mult,
                )
                nc.sync.dma_start(
                    mag_d[:, c0:c0 + n, :], magT[:, c0:c0 + n, :],
                )

        NG = len(GROUPS)
        for j in range(MAX_ITER):
            last = j == MAX_ITER - 1
            z_new = zbuf[j % 2]
            j2 = j + 1

            # early DVE work for iteration j+1 (identities + the b-part of
            # the residual combo; P1(z_{j-1}) is complete, so Rb runs off the
            # critical path — and its place in the DVE queue, ahead of this
            # iteration's zx ops, orders it before the A-chain bank clear)
            if not last:
                aI = small.tile([P, P], F32R, tag="aI")
                nc.vector.tensor_scalar_mul(aI[:], idn[:], alphas[j2])
                bI = None
                if j2 >= 2:
                    bI = small.tile([P, P], F32R, tag="bI")
                    nc.vector.tensor_scalar_mul(bI[:], idn[:], betas[j2])
                ab_tiles[j2] = (aI, bI)
                if j2 >= 2:
                    Rb = small.tile([P, 2 * B], F16, tag="Rb")
                    nc.vector.scalar_tensor_tensor(
                        Rb[:], P1all[:, j2 % 2, :], betas[j2] * float(SC),
                        XcN[:], ALU.mult, ALU.add,
                    )
                    rb_tiles[j2] = Rb

            st_prev = None
            for gi, (c0, n) in enumerate(GROUPS):
                # A-chain work first (always-ready PE work), then the
                # just-in-time momentum for this group
                if pending_chunks() >= DEFER_CHUNKS:
                    emit_A(pending.pop(0))
                if (j, gi) not in u_tiles:
                    emit_mom(j, gi)
                u_ps = u_tiles.pop((j, gi))
                emit_grad(j, u_ps, c0, n)
                # stage 2 of the previous group BEFORE stage 1 of this one:
                # ACT queue = [Rsqrt(g-1), Square(g)], DVE queue =
                # [ns(g-1), zx(g-1), m2(g)] — each op ready when reached
                if st_prev is not None:
                    emit_stage2(j, st_prev, z_new, last)
                st_prev = emit_stage1(j, u_ps, gi, c0, n)

            # flush the last group's stage 2 first: it feeds the A-chain
            # tail -> R4, the iteration-boundary critical path
            emit_stage2(j, st_prev, z_new, last)

            # ---- iteration tail: next iteration's first momentum groups
            # interleaved with the A-chain tail
            if last:
                break
            emit_mom(j2, 0)
            if pending:
                emit_A(pending.pop(0))
            emit_mom(j2, 1)
            while pending:
                emit_A(pending.pop(0))

            # R4 residual combo for j+1:  R4 = (a/64)*P1(z_j) + Rb
            R4n = small.tile([P, 2 * B], F16, tag="R4")
            if j2 == 1:
                nc.vector.scalar_tensor_tensor(
                    R4n[:], P1all[:, j % 2, :], alphas[j2] * float(SC),
                    XcN[:], ALU.mult, ALU.add,
                )
            else:
                nc.vector.scalar_tensor_tensor(
                    R4n[:], P1all[:, j % 2, :], alphas[j2] * float(SC),
                    rb_tiles.pop(j2)[:], ALU.mult, ALU.add,
                )
            # R4ns = [-R4_hi | R4_lo] on DVE ts (fp16, tiny; keeps ACT free)
            R4nsn = small.tile([P, 2 * B], F16, tag="R4ns")
            nc.vector.tensor_scalar_mul(R4nsn[:, 0:B], R4n[:, B:2 * B], -1.0)
            nc.vector.tensor_scalar_mul(R4nsn[:, B:2 * B], R4n[:, 0:B], 1.0)
            r4[j2] = (R4n, R4nsn)

    nc.finalize()
    return nc


def prep_host_inputs(x, D):
    """Builds per-core input maps from the full inputs."""
    Dr = np.ascontiguousarray(D.real).astype(np.float32)
    Di = np.ascontiguousarray(D.imag).astype(np.float32)
    W1c = np.concatenate(
        [Dr.T.reshape(NCH, P, T), Di.T.reshape(NCH, P, T)], axis=2
    )
    W1 = np.ascontiguousarray(W1c.transpose(1, 0, 2))
    # gradient weights scaled by 64*step = 1/64 (residual carries the other
    # 1/64), stored fp16
    s2 = np.float32(64.0 * STEP)
    W2a = np.ascontiguousarray(
        np.concatenate([-s2 * Dr, -s2 * Di], axis=0).reshape(P, NCH, P)
    ).astype(np.float16)
    W2b = np.ascontiguousarray(
        np.concatenate([s2 * Di, -s2 * Dr], axis=0).reshape(P, NCH, P)
    ).astype(np.float16)
    idn = np.eye(P, dtype=np.float32)

    in_maps = []
    for i in range(NCORES):
        xs = x[i * B:(i + 1) * B]
        xr = xs[:, 0].astype(np.float32)
        xi = xs[:, 1].astype(np.float32)
        XcN = np.zeros((P, 2 * B), dtype=np.float32)
        XcN[0:T, 0:B] = -xr.T * SC
        XcN[0:T, B:] = -xi.T * SC
        Rns0 = np.zeros((P, 2 * B), dtype=np.float32)
        Rns0[:, 0:B] = -XcN[:, B:2 * B]
        Rns0[:, B:2 * B] = XcN[:, 0:B]
        in_maps.append({
            "W1": W1, "W2a": W2a, "W2b": W2b,
            "XcN": XcN.astype(np.float16), "Rns0": Rns0.astype(np.float16),
            "idn": idn,
        })
    return in_maps


def gather_output(results):
    outs = []
    for i in range(NCORES):
        magT = results[i]["magT"].reshape(P, NCH, B).astype(np.float32)
        outs.append(np.ascontiguousarray(magT.transpose(2, 1, 0)).reshape(B, F))
    mag_all = np.concatenate(outs, axis=0)
    return (mag_all / mag_all.max()).astype(np.float32)


_NC_CACHE = {}


def get_nc():
    if "nc" not in _NC_CACHE:
        _NC_CACHE["nc"] = build_nc()
    return _NC_CACHE["nc"]


def kernel(x, D):
    x = np.asarray(x)
    D = np.asarray(D)
    nc = get_nc()
    in_maps = prep_host_inputs(x, D)
    res = run_bass_kernel_spmd(nc, in_maps, list(range(NCORES)))
    return gather_output(res.results)


if __name__ == "__main__":
    import reference as ref
    inputs = ref.setup_inputs()
    out = kernel(**{k: np.asarray(v) for k, v in inputs.items()})
    print("kernel output", out.shape, out.dtype)


# revision 11
# speedup vs baseline: 1.1943x; 1.1943x over previous
"""Trainium2 Bass kernel for FISTA sparse coding (nn_FISTA_7550552506950).

Strategy (data-parallel over batch, 8 cores x 128 rows):
- State z kept TRANSPOSED [F=4096, B=128] on-chip as a single fp32 tensor,
  split into 32 f-chunks of [128, 256] (real|imag column halves). Everything
  stays SBUF/PSUM resident across all 25 FISTA iterations; HBM traffic is
  only the initial weight/x load and the final magnitude store.
- Precision plan (validated by host-side quantization ablation):
  * z state and the soft-threshold scale factor stay fp32 (their
    quantization compounds O(k^2) through the momentum recursion);
  * the residual R4 and the gradient weights W2a/W2b are fp16, scaled by
    1/64 so values sit mid-range (fp16 matmuls stream ~1.0 cyc/col vs
    fp32r's ~1.1 + 111ns instruction floor);
  * the threshold pipeline t12/m2 is fp16 SCALED by K=1/(2*thr): unscaled
    |u|^2 ~ 1e-8 sits in fp16's subnormal range and destroys convergence.
    q = Rsqrt(4*m2') = thr/|u| exactly, computed via the raw-Rsqrt escape.
- Per iteration the tensor engine does 40960 column-cycles: 16384 momentum
  (fp32r scaled-identity matmuls folding w = a*z + b*z_old into the PSUM
  accumulation), 16384 gradient (fp16), 8192 A-chain (fp32r).
- Soft-threshold per group: Square on ACT (scale K, fp16 out), m2 pair-add
  on DVE fp16 tensor_tensor (~0.33us vs 1.5us on GPSIMD -- this removes
  GPSIMD from the iteration-boundary critical chain entirely), Rsqrt on
  ACT (scale 4, fp32 out), ns = min(q,1)-1 on DVE, z = (u * -1) * ns as
  DVE scalar_tensor_tensor per half.
- The PE clock (HAM gate) halves after a ~3.4us idle window, so the
  schedule keeps every PE gap short: tapered groups [2,4,...,4,2],
  just-in-time momentum emission, deferred A-chain interleaving, and the
  next iteration's first momentum groups filling the end-of-iteration
  drain. The b-part of the residual combo is precomputed at iteration
  start; R4ns is built by DVE ts ops (fp16) right after the R4 combo.
- P1 products live in a persistent 2-slot (1 PSUM bank) tile; readers of
  the previous slot are ordered before the bank clear via DVE program
  order (see emit order comments).
- Iteration 0 (w = 0) skips all momentum matmuls; iteration 1 (gamma = 0)
  skips the b-part. Global max normalization happens on host.
"""

import numpy as np
from contextlib import ExitStack

import concourse.bass as bass
import concourse.mybir as mybir
import concourse.tile as tile
from concourse import bacc
from concourse.bass_utils import run_bass_kernel_spmd

F32 = mybir.dt.float32
F32R = mybir.dt.float32r
F16 = mybir.dt.float16
ALU = mybir.AluOpType
ACTF = mybir.ActivationFunctionType

P = 128          # partitions / f-chunk size
F = 4096         # dictionary size
T = 64           # signal dim
NCH = F // P     # 32 chunks
B = 128          # batch rows per core
NCORES = 8
MAX_ITER = 25
STEP = np.float32(1.0 / F)
THR = np.float32(0.5) * STEP
SC = np.float32(1.0 / 64)          # residual scaling (keeps fp16 mid-range)
KSQ = float(1.0 / (2.0 * float(THR)))   # Square scale: t12 = (u*K)^2
MAGS = float(4.0 * float(THR))          # |u| = 4*thr * m2' * q
DEFER_CHUNKS = 12   # A-chain chunks deferred behind the threshold pipeline
NS_ON_ACT = set()   # groups whose ns runs on ACT (engine balance)

# group taper: (c0, n) — 2 chunks first (early PSUM release), 2 last (short
# drain chain); 4-chunk groups in between
GROUPS = [(0, 2)] + [(2 + 4 * i, 4) for i in range(7)] + [(30, 2)]


def _activation_raw(nc, out, in_, func, bias, scale=1.0):
    """nc.scalar.activation minus the Rsqrt accuracy guard.

    Safe here: rsqrt feeds the soft-threshold scale factor (error attenuated
    by thr/mag) and the final magnitude (relative error ~1e-3, far inside
    the 2e-2 gate).
    """
    inputs = [nc.scalar.lower_ap(in_)]
    for arg in (bias, scale, 0.0):
        if isinstance(arg, float):
            inputs.append(mybir.ImmediateValue(dtype=F32, value=arg))
        else:
            inputs.append(nc.scalar.lower_ap(arg))
    return nc.scalar.add_instruction(
        mybir.InstActivation(
            name=nc.get_next_instruction_name(),
            func=func,
            ins=inputs,
            outs=[nc.scalar.lower_ap(out)],
        )
    )


def _momentum_scalars():
    ts_ = [1.0]
    for _ in range(MAX_ITER + 1):
        ts_.append((1.0 + np.sqrt(1.0 + 4.0 * ts_[-1] ** 2)) / 2.0)
    alphas, betas = [], []
    for j in range(1, MAX_ITER + 1):
        gam = 0.0 if j <= 2 else (ts_[j - 2] - 1.0) / ts_[j - 1]
        alphas.append(float(np.float32(1.0 + gam)))
        betas.append(float(np.float32(-gam)))
    return alphas, betas


def build_nc():
    nc = bacc.Bacc(None)
    W1_d = nc.declare_dram_parameter("W1", [P, NCH, P], F32R, isOutput=False)
    W2a_d = nc.declare_dram_parameter("W2a", [P, NCH, P], F16, isOutput=False)
    W2b_d = nc.declare_dram_parameter("W2b", [P, NCH, P], F16, isOutput=False)
    XcN_d = nc.declare_dram_parameter("XcN", [P, 2 * B], F16, isOutput=False)
    Rns0_d = nc.declare_dram_parameter("Rns0", [P, 2 * B], F16, isOutput=False)
    idn_d = nc.declare_dram_parameter("idn", [P, P], F32, isOutput=False)
    mag_d = nc.declare_dram_parameter("magT", [P, NCH, B], F16, isOutput=True)

    alphas, betas = _momentum_scalars()

    with tile.TileContext(nc) as tc, ExitStack() as ctx:
        state = ctx.enter_context(tc.tile_pool(name="state", bufs=1))
        temps = ctx.enter_context(tc.tile_pool(name="temps", bufs=3))
        small = ctx.enter_context(tc.tile_pool(name="small", bufs=2))
        psum_u4 = ctx.enter_context(tc.tile_pool(name="psum_u4", bufs=3, space="PSUM"))
        psum_u2 = ctx.enter_context(tc.tile_pool(name="psum_u2", bufs=1, space="PSUM"))
        psum_s = ctx.enter_context(tc.tile_pool(name="psum_s", bufs=1, space="PSUM"))

        # ---- persistent SBUF tensors
        W1 = state.tile([P, NCH, P], F32R, tag="W1")
        W2a = state.tile([P, NCH, P], F16, tag="W2a")
        W2b = state.tile([P, NCH, P], F16, tag="W2b")
        XcN = state.tile([P, 2 * B], F16, tag="XcN")
        Rns0 = state.tile([P, 2 * B], F16, tag="Rns0")
        idn = state.tile([P, P], F32, tag="idn")
        zA = state.tile([P, NCH, 2 * B], F32R, tag="zA")
        zB = state.tile([P, NCH, 2 * B], F32R, tag="zB")
        magT = state.tile([P, NCH, B], F16, tag="magT")
        zero_col = state.tile([P, 1], F32, tag="zc")
        one_col = state.tile([P, 1], F32, tag="oc")
        eps_col = state.tile([P, 1], F32, tag="ec")

        # persistent 2-slot P1 accumulator (1 PSUM bank, slot j%2)
        P1all = psum_s.tile([P, 2, 2 * B], F32, tag="P1")

        nc.sync.dma_start(XcN[:], XcN_d[:])
        nc.sync.dma_start(Rns0[:], Rns0_d[:])
        nc.sync.dma_start(idn[:], idn_d[:])
        for k in range(8):
            cs = slice(4 * k, 4 * k + 4)
            nc.sync.dma_start(W2a[:, cs, :], W2a_d[:, cs, :])
            nc.sync.dma_start(W2b[:, cs, :], W2b_d[:, cs, :])
        nc.sync.dma_start(W1[:], W1_d[:])

        nc.vector.memset(zero_col[:], 0.0)
        nc.vector.memset(one_col[:], 1.0)
        nc.vector.memset(eps_col[:], 1e-30)

        zbuf = [zA, zB]
        pending = []     # deferred A-chain entries: (z_tile, slot, c0, n)
        u_tiles = {}     # (iteration, group-idx) -> u PSUM tile
        r4 = {0: (XcN, Rns0)}   # iteration -> (R4, R4ns), prepared one ahead
        ab_tiles = {}    # iteration -> (aI, bI)
        rb_tiles = {}    # iteration -> Rb = b*P1(z_{j-2}) + XcN, precomputed

        def emit_mom(j, gi):
            """Momentum identity matmuls for iteration j, group gi (allocates
            the group's u PSUM tile: the last group uses the 1-bank pool)."""
            c0, n = GROUPS[gi]
            if gi == len(GROUPS) - 1:
                u_ps = psum_u2.tile([P, 2, 2 * B], F32, tag="u2")
            else:
                u_ps = psum_u4.tile([P, 4, 2 * B], F32, tag="u4")
            u_tiles[(j, gi)] = u_ps
            if j == 0:
                return u_ps
            aI, bI = ab_tiles[j]
            z_prev = zbuf[(j + 1) % 2]
            z_prev2 = zbuf[j % 2]
            for pi in range(n // 2):
                c2 = c0 + 2 * pi
                out_sl = u_ps[:, 2 * pi:2 * pi + 2, :].rearrange("p c n -> p (c n)")
                nc.tensor.matmul(
                    out_sl, aI[:],
                    z_prev[:, c2:c2 + 2, :].rearrange("p c n -> p (c n)"),
                    start=True, stop=False, skip_group_check=True,
                )
                if j >= 2:
                    nc.tensor.matmul(
                        out_sl, bI[:],
                        z_prev2[:, c2:c2 + 2, :].rearrange("p c n -> p (c n)"),
                        start=False, stop=False, skip_group_check=True,
                    )
            return u_ps

        def emit_grad(j, u_ps, c0, n):
            """Gradient matmuls for chunks [c0, c0+n): all W2a first (they
            need only R4, so the next iteration's gradient starts as soon as
            the R4 combo lands; R4ns arrives while the W2a matmuls stream)."""
            R4, R4ns = r4[j]
            mom_on = j >= 1
            for i in range(n):
                nc.tensor.matmul(
                    u_ps[:, i, :], W2a[:, c0 + i, :], R4[:],
                    start=(not mom_on and i % 2 == 0),
                    stop=False, skip_group_check=True,
                )
            for i in range(n):
                nc.tensor.matmul(
                    u_ps[:, i, :], W2b[:, c0 + i, :], R4ns[:],
                    start=False, stop=(i == n - 1), skip_group_check=True,
                )

        def emit_A(entry):
            z_t, slot, c0, n = entry
            for i in range(n):
                c = c0 + i
                nc.tensor.matmul(
                    P1all[:, slot, :], W1[:, c, :], z_t[:, c, :],
                    start=(c == 0), stop=(c == NCH - 1), skip_group_check=True,
                )

        def pending_chunks():
            return sum(e[3] for e in pending)

        def emit_stage1(j, u_ps, gi, c0, n):
            """Threshold stage 1: t12 = (u*K)^2 on ACT (fp16, scaled to
            dodge fp16 subnormals) + m2 pair-add on DVE fp16."""
            u_sl = u_ps[:, 0:n, :]
            t12 = temps.tile([P, n, 2 * B], F16, tag=f"t12_{n}")
            nc.scalar.activation(t12[:], u_sl, ACTF.Square, bias=zero_col[:],
                                 scale=KSQ)
            m2 = temps.tile([P, n, B], F16, tag=f"m2_{n}")
            nc.vector.tensor_tensor(m2[:], t12[:, :, 0:B], t12[:, :, B:2 * B],
                                    ALU.add)
            return (u_ps, gi, c0, n, m2)

        def emit_stage2(j, st, z_new, last):
            """Threshold stage 2 (one group SKEWED behind stage 1 so each
            engine's queue order matches input readiness — no head-of-line
            blocking): q = thr/|u| via Rsqrt(4*m2'), ns = min(q,1)-1 = -s,
            z = (u * -1) * ns."""
            u_ps, gi, c0, n, m2 = st
            u_sl = u_ps[:, 0:n, :]
            q = temps.tile([P, n, B], F32, tag=f"q_{n}")
            _activation_raw(nc, q[:], m2[:], ACTF.Rsqrt, bias=eps_col[:],
                            scale=4.0)
            ns = temps.tile([P, n, B], F32, tag=f"ns_{n}")
            nc.vector.tensor_scalar(ns[:], q[:], 1.0, 1.0,
                                    ALU.min, ALU.subtract)

            if not last:
                z_sl = z_new[:, c0:c0 + n, :]
                nc.vector.scalar_tensor_tensor(
                    z_sl[:, :, 0:B], u_sl[:, :, 0:B], -1.0, ns[:],
                    ALU.mult, ALU.mult,
                )
                nc.vector.scalar_tensor_tensor(
                    z_sl[:, :, B:2 * B], u_sl[:, :, B:2 * B], -1.0, ns[:],
                    ALU.mult, ALU.mult,
                )
                pending.append((z_new, j % 2, c0, n))
            else:
                # |z| = |u|*s = (4*thr * m2' * q) * s  — no Sqrt needed
                tm = temps.tile([P, n, B], F32, tag=f"tm_{n}")
                nc.vector.tensor_tensor(tm[:], m2[:], q[:], ALU.mult)
                nc.vector.scalar_tensor_tensor(
                    magT[:, c0:c0 + n, :], tm[:], -MAGS,
                    ns[:], ALU.mult, ALU.mult,
                )
                nc.sync.dma_start(
                    mag_d[:, c0:c0 + n, :], magT[:, c0:c0 + n, :],
                )

        NG = len(GROUPS)
        for j in range(MAX_ITER):
            last = j == MAX_ITER - 1
            z_new = zbuf[j % 2]
            j2 = j + 1

            # early DVE work for iteration j+1 (identities + the b-part of
            # the residual combo; P1(z_{j-1}) is complete, so Rb runs off the
            # critical path — and its place in the DVE queue, ahead of this
            # iteration's zx ops, orders it before the A-chain bank clear)
            if not last:
                aI = small.tile([P, P], F32R, tag="aI")
                nc.vector.tensor_scalar_mul(aI[:], idn[:], alphas[j2])
                bI = None
                if j2 >= 2:
                    bI = small.tile([P, P], F32R, tag="bI")
                    nc.vector.tensor_scalar_mul(bI[:], idn[:], betas[j2])
                ab_tiles[j2] = (aI, bI)
                if j2 >= 2:
                    Rb = small.tile([P, 2 * B], F16, tag="Rb")
                    nc.vector.scalar_tensor_tensor(
                        Rb[:], P1all[:, j2 % 2, :], betas[j2] * float(SC),
                        XcN[:], ALU.mult, ALU.add,
                    )
                    rb_tiles[j2] = Rb

            for gi, (c0, n) in enumerate(GROUPS):
                # A-chain work first (always-ready PE work), then the
                # just-in-time momentum for this group
                if pending_chunks() >= DEFER_CHUNKS:
                    emit_A(pending.pop(0))
                if (j, gi) not in u_tiles:
                    emit_mom(j, gi)
                u_ps = u_tiles.pop((j, gi))
                emit_grad(j, u_ps, c0, n)
                emit_stage2(j, emit_stage1(j, u_ps, gi, c0, n), z_new, last)

            # ---- iteration tail: next iteration's first momentum groups
            # interleaved with the A-chain tail
            if last:
                break
            emit_mom(j2, 0)
            if pending:
                emit_A(pending.pop(0))
            emit_mom(j2, 1)
            while pending:
                emit_A(pending.pop(0))

            # R4 residual combo for j+1:  R4 = (a/64)*P1(z_j) + Rb
            R4n = small.tile([P, 2 * B], F16, tag="R4")
            if j2 == 1:
                nc.vector.scalar_tensor_tensor(
                    R4n[:], P1all[:, j % 2, :], alphas[j2] * float(SC),
                    XcN[:], ALU.mult, ALU.add,
                )
            else:
                nc.vector.scalar_tensor_tensor(
                    R4n[:], P1all[:, j % 2, :], alphas[j2] * float(SC),
                    rb_tiles.pop(j2)[:], ALU.mult, ALU.add,
                )
            # R4ns = [-R4_hi | R4_lo] on DVE ts (fp16, tiny; keeps ACT free)
            R4nsn = small.tile([P, 2 * B], F16, tag="R4ns")
            nc.vector.tensor_scalar_mul(R4nsn[:, 0:B], R4n[:, B:2 * B], -1.0)
            nc.vector.tensor_scalar_mul(R4nsn[:, B:2 * B], R4n[:, 0:B], 1.0)
            r4[j2] = (R4n, R4nsn)

    nc.finalize()
    return nc


def prep_host_inputs(x, D):
    """Builds per-core input maps from the full inputs."""
    Dr = np.ascontiguousarray(D.real).astype(np.float32)
    Di = np.ascontiguousarray(D.imag).astype(np.float32)
    W1c = np.concatenate(
        [Dr.T.reshape(NCH, P, T), Di.T.reshape(NCH, P, T)], axis=2
    )
    W1 = np.ascontiguousarray(W1c.transpose(1, 0, 2))
    # gradient weights scaled by 64*step = 1/64 (residual carries the other
    # 1/64), stored fp16
    s2 = np.float32(64.0 * STEP)
    W2a = np.ascontiguousarray(
        np.concatenate([-s2 * Dr, -s2 * Di], axis=0).reshape(P, NCH, P)
    ).astype(np.float16)
    W2b = np.ascontiguousarray(
        np.concatenate([s2 * Di, -s2 * Dr], axis=0).reshape(P, NCH, P)
    ).astype(np.float16)
    idn = np.eye(P, dtype=np.float32)

    in_maps = []
    for i in range(NCORES):
        xs = x[i * B:(i + 1) * B]
        xr = xs[:, 0].astype(np.float32)
        xi = xs[:, 1].astype(np.float32)
        XcN = np.zeros((P, 2 * B), dtype=np.float32)
        XcN[0:T, 0:B] = -xr.T * SC
        XcN[0:T, B:] = -xi.T * SC
        Rns0 = np.zeros((P, 2 * B), dtype=np.float32)
        Rns0[:, 0:B] = -XcN[:, B:2 * B]
        Rns0[:, B:2 * B] = XcN[:, 0:B]
        in_maps.append({
            "W1": W1, "W2a": W2a, "W2b": W2b,
            "XcN": XcN.astype(np.float16), "Rns0": Rns0.astype(np.float16),
            "idn": idn,
        })
    return in_maps


def gather_output(results):
    outs = []
    for i in range(NCORES):
        magT = results[i]["magT"].reshape(P, NCH, B).astype(np.float32)
        outs.append(np.ascontiguousarray(magT.transpose(2, 1, 0)).reshape(B, F))
    mag_all = np.concatenate(outs, axis=0)
    return (mag_all / mag_all.max()).astype(np.float32)


_NC_CACHE = {}


def get_nc():
    if "nc" not in _NC_CACHE:
        _NC_CACHE["nc"] = build_nc()
    return _NC_CACHE["nc"]


def kernel(x, D):
    x = np.asarray(x)
    D = np.asarray(D)
    nc = get_nc()
    in_maps = prep_host_inputs(x, D)
    res = run_bass_kernel_spmd(nc, in_maps, list(range(NCORES)))
    return gather_output(res.results)


if __name__ == "__main__":
    import reference as ref
    inputs = ref.setup_inputs()
    out = kernel(**{k: np.asarray(v) for k, v in inputs.items()})
    print("kernel output", out.shape, out.dtype)
